# revision 19
# baseline (speedup 1.0000x reference)
"""Trainium2 Bass kernel for nn_Decoder_24541443129406.

Math: the reference's pdf/pdf_max cancels the normalization, so

    prob[n] = clip( sum_m exp( -0.5 * sum_d (pos[n,d]-mean[m,d])^2 / sigma[m,d] ), 0, 1 )

with pos = [ox, oy, dx, dy], sigma = [sx, sy, 1e-3, 1e-3],
sx = relu(l4)+0.01, sy = relu(l5)+0.01, mean = latents[:, :4].

The exponent is a quadratic form -> a K=8 matmul:
    e[n,m] = f[n] . w[m]
    f[n] = [dx^2+dy^2, 1, ox, oy, dx, dy, ox^2, oy^2]
    w[m] = [c7, c0, c1, c2, c3, c4, c5, c6]
      c1 = mx/sx, c2 = my/sy, c3 = 1000*mdx, c4 = 1000*mdy,
      c5 = -0.5/sx, c6 = -0.5/sy, c7 = -500,
      c0 = -0.5*(mx^2/sx + my^2/sy + 1000*(mdx^2+mdy^2))
emulated at fp32-ish accuracy with one K=24 fp16 matmul of hi/lo split
operands: e = h.H + l.H + h.L (features stacked [h; l; h], weights
[H; H; L]).

Sparsity: sigma_dir = 1e-3 makes the direction factor exp(-500*|d-md|^2)
vanish (< e^-15) unless |d - md| <= sqrt(15/500) ~ 0.173.  The host
culls rays with no gaussian in reach, Morton-sorts the survivors by
direction cell, and packs them into 512-ray windows whose union of
in-reach gaussians is <= 126.  Each window's weight table is the union's
columns (padded with null columns whose only effect is e = -30).
Summing a window's full 128 gaussian rows then equals the full sum over
all 512 gaussians to within 512*e^-15 ~ 1.6e-4.

Device pipeline per window (gaussians on partitions, rays on free dim):
    matmul  e[128g, 512r]  = Wt[24, 128g]^T @ feat[24, 512r]   (PE)
    exp     ex[128g, 512r] = Exp(e)  fp16                      (ACT)
    matmul  s[1, 512r]     = ones[128, 1]^T @ ex               (PE)
    dma     prob[512r]    <- s                                  (PSUM->DRAM)
No vector-engine work and no transposes; the host inverse-permutes,
writes zeros for culled rays, and applies the final clip.
"""

import os
import sys

import numpy as np

for _p in ("/opt/trn_rl_repo", "/root/.axon_site/_ro/trn_rl_repo"):
    if os.path.isdir(_p) and _p not in sys.path:
        sys.path.insert(0, _p)

import concourse.bacc as bacc
import concourse.mybir as mybir
import concourse.tile as tile
from concourse import bass_utils

N_CORES = 8
N = 65536
M = 512
F = 512              # rays per window (one PSUM bank wide)
GPW = 4              # windows vertically packed per PSUM group (4 x 32 rows)
GSLOT = 128 // GPW   # gaussian slots per window
UMAX = GSLOT         # max gaussians unioned per window
TAU = 15.0           # drop pairs with exponent < -TAU  (512*e^-15 ~ 1.6e-4)
REACH = float(np.sqrt(TAU / 500.0))
DELTA = 0.125        # direction-space cell size for sorting/unions
SIGMA_EPS = 0.01
NULL_C0 = -30.0      # padded weight columns produce e = -30 -> exp ~ 1e-13

F32 = mybir.dt.float32
F16 = mybir.dt.float16

TRACE = False
LAST_PERF = None
_CACHED_NC = {}


# --------------------------------------------------------------------------
# device kernel
# --------------------------------------------------------------------------

def build_kernel_body(nc, featcs, wg, prob, ngrp):
    """featcs [24, ngrp*GPW*F] f16, wg [ngrp*24, 128] f16,
    prob [ngrp*GPW*F, 1] f32.  Each group packs GPW windows of F rays
    vertically: window j owns gaussian-slot rows 32j..32j+31."""
    with tile.TileContext(nc) as tc:
        with tc.tile_pool(name="singles", bufs=1) as singles, \
             tc.tile_pool(name="expool", bufs=4) as expool, \
             tc.tile_pool(name="opool", bufs=4) as opool, \
             tc.tile_pool(name="epsum", bufs=4, space="PSUM") as epsum, \
             tc.tile_pool(name="rpsum", bufs=4, space="PSUM") as rpsum:
            probv = prob.rearrange("(s f) o -> s (f o)", f=GPW * F)

            # block-diagonal ones: bd[p, j] = 1 iff p // GSLOT == j
            bd = singles.tile([128, GPW], F16)
            nc.vector.memset(bd, 0.0)
            for j in range(GPW):
                nc.vector.memset(bd[GSLOT * j : GSLOT * (j + 1), j : j + 1], 1.0)

            # Input loads ride gpsimd (free: scalar is blocked by the hoisted
            # ACT table load, sync is saved for outputs; gpsimd's software
            # ring only has a slow teardown drain when DMAs are still in
            # flight, and these all complete early) + sync.  Window 0's
            # features and the weights lead so the first matmul fires early.
            feat = singles.tile([24, ngrp * GPW * F], F16)
            wgall = singles.tile([24, ngrp * 128], F16)
            nc.gpsimd.dma_start(out=wgall, in_=wg)
            nc.sync.dma_start(out=feat[:, 0:F], in_=featcs[:, 0:F])
            nc.gpsimd.dma_start(
                out=feat[:, F : GPW * F], in_=featcs[:, F : GPW * F]
            )
            for s in range(1, ngrp):
                lo, hi = GPW * F * s, GPW * F * (s + 1)
                e = nc.gpsimd if s % 2 else nc.sync
                e.dma_start(out=feat[:, lo:hi], in_=featcs[:, lo:hi])

            for s in range(ngrp):
                ps = epsum.tile([128, F], F32, tag="ps")
                for j in range(GPW):
                    w = GPW * s + j
                    nc.tensor.matmul(
                        out=ps[GSLOT * j : GSLOT * (j + 1), :],
                        lhsT=wgall[:, 128 * s + GSLOT * j : 128 * s + GSLOT * (j + 1)],
                        rhs=feat[:, F * w : F * (w + 1)],
                        start=True,
                        stop=True,
                        tile_position=(0, GSLOT * j),
                    )
                ex = expool.tile([128, F], F16, tag="ex")
                nc.scalar.activation(
                    out=ex, in_=ps, func=mybir.ActivationFunctionType.Exp
                )
                rp = rpsum.tile([GPW, F], F32, tag="rp")
                nc.tensor.matmul(out=rp, lhsT=bd, rhs=ex, start=True, stop=True)
                # PSUM is not DMA-able: clip + move to SBUF on the idle DVE
                rs = opool.tile([GPW, F], F32, tag="rs")
                nc.vector.tensor_scalar(
                    out=rs,
                    in0=rp,
                    scalar1=1.0,
                    scalar2=None,
                    op0=mybir.AluOpType.min,
                )
                e = nc.scalar if s % 2 else nc.sync
                e.dma_start(out=probv[s : s + 1, :], in_=rs)


def build_nc(ngrp):
    nc = bacc.Bacc("TRN2", target_bir_lowering=False, debug=False)
    ncap = ngrp * GPW * F
    featcs = nc.dram_tensor("featcs", [24, ncap], F16, kind="ExternalInput").ap()
    wg = nc.dram_tensor("wg", [24, ngrp * 128], F16, kind="ExternalInput").ap()
    prob = nc.dram_tensor("prob", [ncap, 1], F32, kind="ExternalOutput").ap()
    build_kernel_body(nc, featcs, wg, prob, ngrp)
    nc.compile()
    return nc


# --------------------------------------------------------------------------
# host-side binning / packing
# --------------------------------------------------------------------------

def _morton_key(ci):
    x = (ci[:, 0] + 2048).astype(np.uint64)
    y = (ci[:, 1] + 2048).astype(np.uint64)
    k = np.zeros_like(x)
    for b in range(12):
        k |= ((x >> np.uint64(b)) & np.uint64(1)) << np.uint64(2 * b)
        k |= ((y >> np.uint64(b)) & np.uint64(1)) << np.uint64(2 * b + 1)
    return k


def _weights(latents):
    """Exact per-gaussian weight rows [8, M] float32 in feature order."""
    lat = latents.astype(np.float64)
    mx, my, mdx, mdy = lat[:, 0], lat[:, 1], lat[:, 2], lat[:, 3]
    sx = np.maximum(lat[:, 4], 0.0) + SIGMA_EPS
    sy = np.maximum(lat[:, 5], 0.0) + SIGMA_EPS
    c0 = -0.5 * (mx * mx / sx + my * my / sy + 1000.0 * (mdx * mdx + mdy * mdy))
    w = np.stack(
        [
            np.full_like(c0, -500.0),  # * (dx^2+dy^2)
            c0,                        # * 1
            mx / sx,                   # * ox
            my / sy,                   # * oy
            1000.0 * mdx,              # * dx
            1000.0 * mdy,              # * dy
            -0.5 / sx,                 # * ox^2
            -0.5 / sy,                 # * oy^2
        ],
        axis=0,
    )
    return w


def _plan(directions):
    """Cull + sort + window-pack rays.  Returns (sorted_idx, windows) where
    windows is a list of (n_rays, gauss_index_array); sorted_idx lists the
    device rays in window order (concatenated, unpadded)."""
    d = directions.astype(np.float32)
    ci_all = np.floor(d / DELTA).astype(np.int64)

    # occupied cells and their in-reach gaussian sets (rect distance)
    cells, inv = np.unique(ci_all, axis=0, return_inverse=True)
    lo = cells * DELTA
    hi = lo + DELTA
    ddx = np.maximum(np.maximum(lo[:, 0:1] - _MD[:, 0], _MD[:, 0] - hi[:, 0:1]), 0.0)
    ddy = np.maximum(np.maximum(lo[:, 1:2] - _MD[:, 1], _MD[:, 1] - hi[:, 1:2]), 0.0)
    cell_hits = ddx * ddx + ddy * ddy <= REACH * REACH  # [n_cells, M]

    # exact per-ray cull using the cell's candidate set
    keep = np.zeros(len(d), dtype=bool)
    for c in range(len(cells)):
        gs = np.nonzero(cell_hits[c])[0]
        if len(gs) == 0:
            continue
        rows = np.nonzero(inv == c)[0]
        dd = d[rows]
        dist2 = (dd[:, 0:1] - _MD[gs, 0]) ** 2 + (dd[:, 1:2] - _MD[gs, 1]) ** 2
        keep[rows] = (dist2 <= REACH * REACH).any(axis=1)

    kept = np.nonzero(keep)[0]
    order = np.argsort(_morton_key(ci_all[kept]), kind="stable")
    sorted_idx = kept[order]

    # walk cell runs in sorted order, pack into windows
    cell_of = inv[sorted_idx]
    windows = []
    cur_mask = np.zeros(M, dtype=bool)
    cur_n = 0
    i = 0
    n_dev = len(sorted_idx)
    while i < n_dev:
        c = cell_of[i]
        j = i
        while j < n_dev and cell_of[j] == c:
            j += 1
        run = j - i
        gmask = cell_hits[c]
        while run > 0:
            nu = np.count_nonzero(cur_mask | gmask)
            if cur_n > 0 and (nu > UMAX or cur_n == F):
                windows.append((cur_n, np.nonzero(cur_mask)[0]))
                cur_mask = np.zeros(M, dtype=bool)
                cur_n = 0
                continue
            assert nu <= UMAX, f"single cell union {nu} > {UMAX}"
            take = min(F - cur_n, run)
            cur_mask |= gmask
            cur_n += take
            run -= take
        i = j
    if cur_n > 0:
        windows.append((cur_n, np.nonzero(cur_mask)[0]))
    return sorted_idx, windows


_MD = None  # gaussian direction means, set per call


def kernel(origins: np.ndarray, directions: np.ndarray, latents: np.ndarray) -> np.ndarray:
    global _CACHED_NC, LAST_PERF, _MD
    assert origins.shape == (N, 2) and directions.shape == (N, 2)
    assert latents.shape == (M, 6)
    origins = np.ascontiguousarray(origins, dtype=np.float32)
    directions = np.ascontiguousarray(directions, dtype=np.float32)
    latents = np.ascontiguousarray(latents, dtype=np.float32)

    _MD = latents[:, 2:4].astype(np.float32)
    sorted_idx, windows = _plan(directions)
    n_w = len(windows)
    ngrp = max(1, -(-n_w // (N_CORES * GPW)))  # groups per core, ceil
    nwc = ngrp * GPW                           # window slots per core

    # ---- weights: fp16 hi/lo, stacked [H; H; L] rows ----
    w64 = _weights(latents)  # [8, M] float64
    H = w64.astype(np.float16)
    L = (w64 - H.astype(np.float64)).astype(np.float16)
    null_col = np.zeros((24,), dtype=np.float16)
    null_col[1] = NULL_C0  # c0 row of H
    null_col[9] = NULL_C0  # duplicated H block

    # one [24, 128] table per group; window j owns columns 32j..32j+31
    wg_all = np.tile(
        null_col[None, :, None], (N_CORES * ngrp, 1, 128)
    ).astype(np.float16)
    for wi, (_, gidx) in enumerate(windows):
        u = len(gidx)
        g, j = divmod(wi, GPW)
        wg_all[g, 0:8, GSLOT * j : GSLOT * j + u] = H[:, gidx]
        wg_all[g, 8:16, GSLOT * j : GSLOT * j + u] = H[:, gidx]
        wg_all[g, 16:24, GSLOT * j : GSLOT * j + u] = L[:, gidx]
    # device layout: [24, ngrp*128] per core (group tables side by side)
    wg_dev = np.ascontiguousarray(
        wg_all.reshape(N_CORES, ngrp, 24, 128).transpose(0, 2, 1, 3)
    ).reshape(N_CORES, 24, ngrp * 128)

    # ---- features: fp16 hi/lo, stacked [h; l; h] rows, window-packed ----
    ncap = nwc * F
    ox = origins[sorted_idx, 0]
    oy = origins[sorted_idx, 1]
    dx = directions[sorted_idx, 0]
    dy = directions[sorted_idx, 1]
    f32 = np.stack(
        [dx * dx + dy * dy, np.ones_like(ox), ox, oy, dx, dy, ox * ox, oy * oy],
        axis=0,
    ).astype(np.float32)  # [8, n_dev]
    h = f32.astype(np.float16)
    l = (f32 - h.astype(np.float32)).astype(np.float16)

    feat_all = np.zeros((N_CORES, 24, ncap), dtype=np.float16)
    # scatter rays into their window slots
    pos = 0
    slot_of_ray = np.empty(len(sorted_idx), dtype=np.int64)
    for wi, (n_rays, _) in enumerate(windows):
        core, s = divmod(wi, nwc)
        base = s * F
        sl = np.arange(n_rays)
        slot_of_ray[pos : pos + n_rays] = core * ncap + base + sl
        pos += n_rays
    assert pos == len(sorted_idx)
    core_ids = slot_of_ray // ncap
    local = slot_of_ray % ncap
    feat_all[core_ids, :, local] = np.concatenate([h, l, h], axis=0).T

    key = ngrp
    if key not in _CACHED_NC:
        _CACHED_NC[key] = build_nc(ngrp)
    nc = _CACHED_NC[key]

    in_maps = []
    for c in range(N_CORES):
        in_maps.append(
            {
                "featcs": np.ascontiguousarray(feat_all[c]),
                "wg": np.ascontiguousarray(wg_dev[c]),
            }
        )

    results = bass_utils.run_bass_kernel_spmd(
        nc,
        in_maps,
        core_ids=list(range(N_CORES)),
        trace=TRACE,
    )
    LAST_PERF = results

    dev = np.concatenate(
        [results.results[c]["prob"].reshape(-1) for c in range(N_CORES)]
    )  # [N_CORES * ncap]
    flat_slots = core_ids * ncap + local
    out = np.zeros(N, dtype=np.float32)
    out[sorted_idx] = dev[flat_slots]
    np.clip(out, 0.0, 1.0, out=out)
    return out.reshape(-1, 1).astype(np.float32)


if __name__ == "__main__":
    rng = np.random.default_rng(0)
    o = rng.standard_normal((N, 2), dtype=np.float32)
    d = rng.standard_normal((N, 2), dtype=np.float32)
    l = rng.standard_normal((M, 6), dtype=np.float32)
    p = kernel(o, d, l)
    print(p.shape, p.dtype, p.min(), p.max())


# revision 20
# speedup vs baseline: 1.1360x; 1.1360x over previous
"""Trainium2 Bass kernel for nn_Decoder_24541443129406.

Math: the reference's pdf/pdf_max cancels the normalization, so

    prob[n] = clip( sum_m exp( -0.5 * sum_d (pos[n,d]-mean[m,d])^2 / sigma[m,d] ), 0, 1 )

with pos = [ox, oy, dx, dy], sigma = [sx, sy, 1e-3, 1e-3],
sx = relu(l4)+0.01, sy = relu(l5)+0.01, mean = latents[:, :4].

The exponent is a quadratic form -> a K=8 matmul:
    e[n,m] = f[n] . w[m]
    f[n] = [dx^2+dy^2, 1, ox, oy, dx, dy, ox^2, oy^2]
    w[m] = [c7, c0, c1, c2, c3, c4, c5, c6]
      c1 = mx/sx, c2 = my/sy, c3 = 1000*mdx, c4 = 1000*mdy,
      c5 = -0.5/sx, c6 = -0.5/sy, c7 = -500,
      c0 = -0.5*(mx^2/sx + my^2/sy + 1000*(mdx^2+mdy^2))
emulated at fp32-ish accuracy with one K=24 fp16 matmul of hi/lo split
operands: e = h.H + l.H + h.L (features stacked [h; l; h], weights
[H; H; L]).

Sparsity: sigma_dir = 1e-3 makes the direction factor exp(-500*|d-md|^2)
vanish (< e^-15) unless |d - md| <= sqrt(15/500) ~ 0.173.  The host
culls rays with no gaussian in reach, Morton-sorts the survivors by
direction cell, and packs them into 512-ray windows whose union of
in-reach gaussians is <= 126.  Each window's weight table is the union's
columns (padded with null columns whose only effect is e = -30).
Summing a window's full 128 gaussian rows then equals the full sum over
all 512 gaussians to within 512*e^-15 ~ 1.6e-4.

Device pipeline per window (gaussians on partitions, rays on free dim):
    matmul  e[128g, 512r]  = Wt[24, 128g]^T @ feat[24, 512r]   (PE)
    exp     ex[128g, 512r] = Exp(e)  fp16                      (ACT)
    matmul  s[1, 512r]     = ones[128, 1]^T @ ex               (PE)
    dma     prob[512r]    <- s                                  (PSUM->DRAM)
No vector-engine work and no transposes; the host inverse-permutes,
writes zeros for culled rays, and applies the final clip.
"""

import os
import sys

import numpy as np

for _p in ("/opt/trn_rl_repo", "/root/.axon_site/_ro/trn_rl_repo"):
    if os.path.isdir(_p) and _p not in sys.path:
        sys.path.insert(0, _p)

import concourse.bacc as bacc
import concourse.mybir as mybir
import concourse.tile as tile
from concourse import bass_utils

N_CORES = 8
N = 65536
M = 512
F = 512              # rays per window (one PSUM bank wide)
GPW = 4              # windows vertically packed per PSUM group (4 x 32 rows)
GSLOT = 128 // GPW   # gaussian slots per window
UMAX = GSLOT         # max gaussians unioned per window
TAU = 15.0           # drop pairs with exponent < -TAU  (512*e^-15 ~ 1.6e-4)
REACH = float(np.sqrt(TAU / 500.0))
DELTA = 0.125        # direction-space cell size for sorting/unions
SIGMA_EPS = 0.01
NULL_C0 = -30.0      # padded weight columns produce e = -30 -> exp ~ 1e-13

F32 = mybir.dt.float32
F16 = mybir.dt.float16

TRACE = False
LAST_PERF = None
_CACHED_NC = {}


# --------------------------------------------------------------------------
# device kernel
# --------------------------------------------------------------------------

def build_kernel_body(nc, featcs, wg, prob, ngrp):
    """featcs [24, ngrp*GPW*F] f16, wg [ngrp*24, 128] f16,
    prob [ngrp*GPW*F, 1] f32.  Each group packs GPW windows of F rays
    vertically: window j owns gaussian-slot rows 32j..32j+31."""
    with tile.TileContext(nc) as tc:
        with tc.tile_pool(name="singles", bufs=1) as singles, \
             tc.tile_pool(name="expool", bufs=4) as expool, \
             tc.tile_pool(name="opool", bufs=4) as opool, \
             tc.tile_pool(name="epsum", bufs=4, space="PSUM") as epsum, \
             tc.tile_pool(name="rpsum", bufs=4, space="PSUM") as rpsum:
            probv = prob.rearrange("(s f) o -> s (f o)", f=GPW * F)

            # block-diagonal ones: bd[p, j] = 1 iff p // GSLOT == j
            bd = singles.tile([128, GPW], F16)
            nc.vector.memset(bd, 0.0)
            for j in range(GPW):
                nc.vector.memset(bd[GSLOT * j : GSLOT * (j + 1), j : j + 1], 1.0)

            # All DMAs ride the two hardware DGE queues (sync + scalar):
            # gpsimd's software ring adds ~2us completion latency and a slow
            # teardown drain.  Window 0's features and the weights lead so
            # the first matmul fires as early as possible.
            feat = singles.tile([24, ngrp * GPW * F], F16)
            wgall = singles.tile([24, ngrp * 128], F16)
            nc.sync.dma_start(out=feat[:, 0:F], in_=featcs[:, 0:F])
            nc.scalar.dma_start(out=wgall, in_=wg)
            nc.sync.dma_start(
                out=feat[:, F : GPW * F], in_=featcs[:, F : GPW * F]
            )
            for s in range(1, ngrp):
                lo, hi = GPW * F * s, GPW * F * (s + 1)
                e = nc.scalar if s % 2 else nc.sync
                e.dma_start(out=feat[:, lo:hi], in_=featcs[:, lo:hi])

            for s in range(ngrp):
                ps = epsum.tile([128, F], F32, tag="ps")
                for j in range(GPW):
                    w = GPW * s + j
                    nc.tensor.matmul(
                        out=ps[GSLOT * j : GSLOT * (j + 1), :],
                        lhsT=wgall[:, 128 * s + GSLOT * j : 128 * s + GSLOT * (j + 1)],
                        rhs=feat[:, F * w : F * (w + 1)],
                        start=True,
                        stop=True,
                        tile_position=(0, GSLOT * j),
                    )
                ex = expool.tile([128, F], F16, tag="ex")
                nc.scalar.activation(
                    out=ex, in_=ps, func=mybir.ActivationFunctionType.Exp
                )
                rp = rpsum.tile([GPW, F], F32, tag="rp")
                nc.tensor.matmul(out=rp, lhsT=bd, rhs=ex, start=True, stop=True)
                # PSUM is not DMA-able: clip + move to SBUF on the idle DVE
                rs = opool.tile([GPW, F], F32, tag="rs")
                nc.vector.tensor_scalar(
                    out=rs,
                    in0=rp,
                    scalar1=1.0,
                    scalar2=None,
                    op0=mybir.AluOpType.min,
                )
                e = nc.scalar if s % 2 else nc.sync
                e.dma_start(out=probv[s : s + 1, :], in_=rs)


def build_nc(ngrp):
    nc = bacc.Bacc("TRN2", target_bir_lowering=False, debug=False)
    ncap = ngrp * GPW * F
    featcs = nc.dram_tensor("featcs", [24, ncap], F16, kind="ExternalInput").ap()
    wg = nc.dram_tensor("wg", [24, ngrp * 128], F16, kind="ExternalInput").ap()
    prob = nc.dram_tensor("prob", [ncap, 1], F32, kind="ExternalOutput").ap()
    build_kernel_body(nc, featcs, wg, prob, ngrp)
    nc.compile()
    return nc


# --------------------------------------------------------------------------
# host-side binning / packing
# --------------------------------------------------------------------------

def _morton_key(ci):
    x = (ci[:, 0] + 2048).astype(np.uint64)
    y = (ci[:, 1] + 2048).astype(np.uint64)
    k = np.zeros_like(x)
    for b in range(12):
        k |= ((x >> np.uint64(b)) & np.uint64(1)) << np.uint64(2 * b)
        k |= ((y >> np.uint64(b)) & np.uint64(1)) << np.uint64(2 * b + 1)
    return k


def _weights(latents):
    """Exact per-gaussian weight rows [8, M] float32 in feature order."""
    lat = latents.astype(np.float64)
    mx, my, mdx, mdy = lat[:, 0], lat[:, 1], lat[:, 2], lat[:, 3]
    sx = np.maximum(lat[:, 4], 0.0) + SIGMA_EPS
    sy = np.maximum(lat[:, 5], 0.0) + SIGMA_EPS
    c0 = -0.5 * (mx * mx / sx + my * my / sy + 1000.0 * (mdx * mdx + mdy * mdy))
    w = np.stack(
        [
            np.full_like(c0, -500.0),  # * (dx^2+dy^2)
            c0,                        # * 1
            mx / sx,                   # * ox
            my / sy,                   # * oy
            1000.0 * mdx,              # * dx
            1000.0 * mdy,              # * dy
            -0.5 / sx,                 # * ox^2
            -0.5 / sy,                 # * oy^2
        ],
        axis=0,
    )
    return w


def _plan(directions):
    """Cull + sort + window-pack rays.  Returns (sorted_idx, windows) where
    windows is a list of (n_rays, gauss_index_array); sorted_idx lists the
    device rays in window order (concatenated, unpadded)."""
    d = directions.astype(np.float32)
    ci_all = np.floor(d / DELTA).astype(np.int64)

    # occupied cells and their in-reach gaussian sets (rect distance)
    cells, inv = np.unique(ci_all, axis=0, return_inverse=True)
    lo = cells * DELTA
    hi = lo + DELTA
    ddx = np.maximum(np.maximum(lo[:, 0:1] - _MD[:, 0], _MD[:, 0] - hi[:, 0:1]), 0.0)
    ddy = np.maximum(np.maximum(lo[:, 1:2] - _MD[:, 1], _MD[:, 1] - hi[:, 1:2]), 0.0)
    cell_hits = ddx * ddx + ddy * ddy <= REACH * REACH  # [n_cells, M]

    # exact per-ray cull using the cell's candidate set
    keep = np.zeros(len(d), dtype=bool)
    for c in range(len(cells)):
        gs = np.nonzero(cell_hits[c])[0]
        if len(gs) == 0:
            continue
        rows = np.nonzero(inv == c)[0]
        dd = d[rows]
        dist2 = (dd[:, 0:1] - _MD[gs, 0]) ** 2 + (dd[:, 1:2] - _MD[gs, 1]) ** 2
        keep[rows] = (dist2 <= REACH * REACH).any(axis=1)

    kept = np.nonzero(keep)[0]
    order = np.argsort(_morton_key(ci_all[kept]), kind="stable")
    sorted_idx = kept[order]

    # walk cell runs in sorted order, pack into windows
    cell_of = inv[sorted_idx]
    windows = []
    cur_mask = np.zeros(M, dtype=bool)
    cur_n = 0
    i = 0
    n_dev = len(sorted_idx)
    while i < n_dev:
        c = cell_of[i]
        j = i
        while j < n_dev and cell_of[j] == c:
            j += 1
        run = j - i
        gmask = cell_hits[c]
        while run > 0:
            nu = np.count_nonzero(cur_mask | gmask)
            if cur_n > 0 and (nu > UMAX or cur_n == F):
                windows.append((cur_n, np.nonzero(cur_mask)[0]))
                cur_mask = np.zeros(M, dtype=bool)
                cur_n = 0
                continue
            assert nu <= UMAX, f"single cell union {nu} > {UMAX}"
            take = min(F - cur_n, run)
            cur_mask |= gmask
            cur_n += take
            run -= take
        i = j
    if cur_n > 0:
        windows.append((cur_n, np.nonzero(cur_mask)[0]))
    return sorted_idx, windows


_MD = None  # gaussian direction means, set per call


def kernel(origins: np.ndarray, directions: np.ndarray, latents: np.ndarray) -> np.ndarray:
    global _CACHED_NC, LAST_PERF, _MD
    assert origins.shape == (N, 2) and directions.shape == (N, 2)
    assert latents.shape == (M, 6)
    origins = np.ascontiguousarray(origins, dtype=np.float32)
    directions = np.ascontiguousarray(directions, dtype=np.float32)
    latents = np.ascontiguousarray(latents, dtype=np.float32)

    _MD = latents[:, 2:4].astype(np.float32)
    sorted_idx, windows = _plan(directions)
    n_w = len(windows)
    ngrp = max(1, -(-n_w // (N_CORES * GPW)))  # groups per core, ceil
    nwc = ngrp * GPW                           # window slots per core

    # ---- weights: fp16 hi/lo, stacked [H; H; L] rows ----
    w64 = _weights(latents)  # [8, M] float64
    H = w64.astype(np.float16)
    L = (w64 - H.astype(np.float64)).astype(np.float16)
    null_col = np.zeros((24,), dtype=np.float16)
    null_col[1] = NULL_C0  # c0 row of H
    null_col[9] = NULL_C0  # duplicated H block

    # one [24, 128] table per group; window j owns columns 32j..32j+31
    wg_all = np.tile(
        null_col[None, :, None], (N_CORES * ngrp, 1, 128)
    ).astype(np.float16)
    for wi, (_, gidx) in enumerate(windows):
        u = len(gidx)
        g, j = divmod(wi, GPW)
        wg_all[g, 0:8, GSLOT * j : GSLOT * j + u] = H[:, gidx]
        wg_all[g, 8:16, GSLOT * j : GSLOT * j + u] = H[:, gidx]
        wg_all[g, 16:24, GSLOT * j : GSLOT * j + u] = L[:, gidx]
    # device layout: [24, ngrp*128] per core (group tables side by side)
    wg_dev = np.ascontiguousarray(
        wg_all.reshape(N_CORES, ngrp, 24, 128).transpose(0, 2, 1, 3)
    ).reshape(N_CORES, 24, ngrp * 128)

    # ---- features: fp16 hi/lo, stacked [h; l; h] rows, window-packed ----
    ncap = nwc * F
    ox = origins[sorted_idx, 0]
    oy = origins[sorted_idx, 1]
    dx = directions[sorted_idx, 0]
    dy = directions[sorted_idx, 1]
    f32 = np.stack(
        [dx * dx + dy * dy, np.ones_like(ox), ox, oy, dx, dy, ox * ox, oy * oy],
        axis=0,
    ).astype(np.float32)  # [8, n_dev]
    h = f32.astype(np.float16)
    l = (f32 - h.astype(np.float32)).astype(np.float16)

    feat_all = np.zeros((N_CORES, 24, ncap), dtype=np.float16)
    # scatter rays into their window slots
    pos = 0
    slot_of_ray = np.empty(len(sorted_idx), dtype=np.int64)
    for wi, (n_rays, _) in enumerate(windows):
        core, s = divmod(wi, nwc)
        base = s * F
        sl = np.arange(n_rays)
        slot_of_ray[pos : pos + n_rays] = core * ncap + base + sl
        pos += n_rays
    assert pos == len(sorted_idx)
    core_ids = slot_of_ray // ncap
    local = slot_of_ray % ncap
    feat_all[core_ids, :, local] = np.concatenate([h, l, h], axis=0).T

    key = ngrp
    if key not in _CACHED_NC:
        _CACHED_NC[key] = build_nc(ngrp)
    nc = _CACHED_NC[key]

    in_maps = []
    for c in range(N_CORES):
        in_maps.append(
            {
                "featcs": np.ascontiguousarray(feat_all[c]),
                "wg": np.ascontiguousarray(wg_dev[c]),
            }
        )

    results = bass_utils.run_bass_kernel_spmd(
        nc,
        in_maps,
        core_ids=list(range(N_CORES)),
        trace=TRACE,
    )
    LAST_PERF = results

    dev = np.concatenate(
        [results.results[c]["prob"].reshape(-1) for c in range(N_CORES)]
    )  # [N_CORES * ncap]
    flat_slots = core_ids * ncap + local
    out = np.zeros(N, dtype=np.float32)
    out[sorted_idx] = dev[flat_slots]
    np.clip(out, 0.0, 1.0, out=out)
    return out.reshape(-1, 1).astype(np.float32)


if __name__ == "__main__":
    rng = np.random.default_rng(0)
    o = rng.standard_normal((N, 2), dtype=np.float32)
    d = rng.standard_normal((N, 2), dtype=np.float32)
    l = rng.standard_normal((M, 6), dtype=np.float32)
    p = kernel(o, d, l)
    print(p.shape, p.dtype, p.min(), p.max())
